# revision 4
# baseline (speedup 1.0000x reference)
"""Trainium2 Bass kernel for block-neighbor "contamination" stencil.

Problem: x [B=8, C=32, H=512, W=512] f32, kernel_size k=8.
The image is a 64x64 grid of 8x8 blocks. For each block, out = 0.8*block +
0.2 * mean(8 neighboring blocks) elementwise over the 8x8 tile, with
zero-padding of the block grid and per-position valid-neighbor counts
(interior 8, edges 5, corners 3).

Equivalent pixel form: a sparse 3x3 stencil with taps at +-8 pixels:
    out[r,w] = 0.8*x[r,w] + beta(r,w) * nsum[r,w]
    nsum[r,w] = sum over (dr,dw) in {-8,0,8}^2, (dr,dw) != (0,0), of
                x[r+dr, w+dw]  (zero pad at image borders)
    beta(r,w) = 0.2 / count(r,w),  count = Nr*Nw - 1,
    Nr/Nw = 2 at the first/last block row/col, else 3.

Strategy (pure data parallel, 1 batch item per NeuronCore, 8 cores).
The problem is HBM-bound (measured: DMA 92% busy at f32 I/O), so I/O is
done in bf16: the host casts x to bf16 (16 MiB/core in), the kernel
writes bf16 (16 MiB/core out), and the host upcasts to f32. End-to-end
rel err ~2.4e-3 (quantization), well within tolerance. This halves HBM
traffic: 32 MiB/core @ ~358 GB/s -> ~94 us floor vs 187 us at f32.

  * Layout: SBUF partition p = (channel-pair, block-row bi); free dim =
    (u = row-within-block 0..7, w 0..511). One partition = one block-row =
    8 consecutive image rows = 8KB contiguous DRAM at bf16.
  * Vertical block-neighbor taps (rows +-8) land at partition +-1 ->
    TensorEngine banded 128x128 matmuls (block-diagonal per channel),
    PSUM f32 accumulation. Horizontal taps are +-8 shifts along w via
    shifted moving-operand access patterns. beta(row) folded into the
    stationary weights; the center (0,0) tap with weight 0.8 is ALSO
    folded into the unshifted stationary (wc), so PSUM holds the full
    result for interior block-cols and the combine is a pure copy.
  * The copy PSUM->SBUF(bf16) runs on ScalarE (sits closest to PSUM;
    VectorE from PSUM is 1 elem/cycle and would become the bottleneck).
  * w-edge block-cols (first/last 8 columns) need the neighbor term
    rescaled by gamma(row) = (3Nr-1)/(2Nr-1): tiny diagonal matmuls
    (we) accumulate -0.8*(gamma-1)/gamma * x into the edge columns of
    PSUM, so out_edge = gamma * PSUM there -- one per-partition-scaled
    DVE copy (tensor_scalar_mul) per strip.
  * Matmuls are batched per stationary (4x wc, then 8x wf, then 8x we
    per half-tile) so LoadStationary switches hide under the PE's
    reorder window.
  * Input DMAs ride the qSyncDynamicHW ring, output DMAs the
    qScalarDynamicHW ring, so loads and stores stream concurrently.
"""

import numpy as np

import concourse.mybir as mybir
import concourse.tile as tile
from concourse import bacc
from concourse.bass_utils import run_bass_kernel_spmd

import ml_dtypes

_BF16 = ml_dtypes.bfloat16

# Problem constants (hardcoded per harness contract).
B, C, H, W = 8, 32, 512, 512
K = 8  # block size
P = 128  # SBUF partitions
NBR = H // K  # 64 block-rows per channel
CPP = P // NBR  # channels per partition-tile (2)
N_CORES = 8
N_CHUNKS = C // CPP  # 16 tiles per core
HALF = K // 2  # u-slices per PSUM tile (4 banks)

BETA_INT = 0.2 / 8.0  # interior block-rows, interior block-cols
BETA_EDGE = 0.2 / 5.0  # edge block-rows, interior block-cols
GAMMA_INT = 8.0 / 5.0  # count ratio (3*Nr-1)/(2*Nr-1) at Nr=3
GAMMA_EDGE = 5.0 / 3.0  # at Nr=2

_EDGE_PARTS = (0, NBR - 1, NBR, P - 1)  # block-row 0/63 of each channel


def _make_weights():
    """Banded stationary matrices (vertical taps at partition +-1),
    block-diagonal per channel, beta folded in per output partition.
    wc additionally carries the 0.8 center tap; we carries the per-row
    edge-column correction -0.8*(gamma-1)/gamma on the diagonal."""
    beta = np.full(P, BETA_INT, np.float32)
    beta[list(_EDGE_PARTS)] = BETA_EDGE
    gamma = np.full(P, GAMMA_INT, np.float32)
    gamma[list(_EDGE_PARTS)] = GAMMA_EDGE

    wf = np.zeros((P, P), np.float32)  # 3-band: taps for shifted columns
    wc = np.zeros((P, P), np.float32)  # 2-band + 0.8 center: unshifted
    for m in range(P):
        for d in (-1, 0, 1):
            k = m + d
            if 0 <= k < P and k // NBR == m // NBR:
                wf[k, m] = beta[m]
                if d != 0:
                    wc[k, m] = beta[m]
        wc[m, m] = 0.8
    we = np.diag(-0.8 * (gamma - 1.0) / gamma).astype(np.float32)
    gv = gamma.reshape(P, 1)
    return {
        "wf": wf.astype(_BF16),
        "wc": wc.astype(_BF16),
        "we": we.astype(_BF16),
        "gv": gv,
    }


def _build_program():
    f32 = mybir.dt.float32
    bf16 = mybir.dt.bfloat16

    nc = bacc.Bacc("TRN2", target_bir_lowering=False, debug=False,
                   num_devices=N_CORES)

    x_dram = nc.dram_tensor("x", [C, H, W], bf16, kind="ExternalInput")
    y_dram = nc.dram_tensor("y", [C, H, W], bf16, kind="ExternalOutput")
    wf_dram = nc.dram_tensor("wf", [P, P], bf16, kind="ExternalInput")
    wc_dram = nc.dram_tensor("wc", [P, P], bf16, kind="ExternalInput")
    we_dram = nc.dram_tensor("we", [P, P], bf16, kind="ExternalInput")
    gv_dram = nc.dram_tensor("gv", [P, 1], f32, kind="ExternalInput")

    # partition axis = (channel, block-row); free = (u, w)
    x_v = x_dram[:].rearrange("c (bi u) w -> (c bi) u w", u=K)
    y_v = y_dram[:].rearrange("c (bi u) w -> (c bi) u w", u=K)

    with tile.TileContext(nc) as tc:
        with (
            tc.tile_pool(name="wpool", bufs=1) as wpool,
            tc.tile_pool(name="sbuf", bufs=4) as sbuf,
            tc.tile_pool(name="psum", bufs=2, space="PSUM") as psum,
        ):
            wf_t = wpool.tile([P, P], bf16, tag="wf")
            nc.sync.dma_start(wf_t[:], wf_dram[:])
            wc_t = wpool.tile([P, P], bf16, tag="wc")
            nc.sync.dma_start(wc_t[:], wc_dram[:])
            we_t = wpool.tile([P, P], bf16, tag="we")
            nc.sync.dma_start(we_t[:], we_dram[:])
            gv_t = wpool.tile([P, 1], f32, tag="gv")
            nc.sync.dma_start(gv_t[:], gv_dram[:])

            for i in range(N_CHUNKS):
                p0 = i * P
                xin = sbuf.tile([P, K, W], bf16, tag="xin")
                # loads stay exclusively on the qSyncDynamicHW ring:
                # mixing dependent stores into the same FIFO ring
                # head-of-line-blocks later loads
                nc.sync.dma_start(xin[:], x_v[p0 : p0 + P])

                out_t = sbuf.tile([P, K, W], bf16, tag="out")
                for h in range(2):
                    u0 = h * HALF
                    u = psum.tile([P, HALF, W], f32, tag="u")
                    # batched per stationary: wc x4, wf x8, we x8.
                    # each PSUM bank (one per uu) gets start on its wc
                    # matmul and stop on its last we matmul.
                    for uu in range(HALF):
                        # center 0.8 tap + vertical taps, full width
                        nc.tensor.matmul(
                            u[:, uu, :], wc_t[:], xin[:, u0 + uu, :],
                            start=True, stop=False,
                        )
                    for uu in range(HALF):
                        su = u0 + uu
                        # dj=-1 / dj=+1 neighbor block-columns
                        nc.tensor.matmul(
                            u[:, uu, K:W], wf_t[:], xin[:, su, : W - K],
                            start=False, stop=False,
                        )
                        nc.tensor.matmul(
                            u[:, uu, : W - K], wf_t[:], xin[:, su, K:W],
                            start=False, stop=False,
                        )
                    # edge-column correction so out_edge = gamma*PSUM;
                    # one matmul per side covers all 4 u-slices via a
                    # 2-D moving access pattern (FD=32)
                    nc.tensor.matmul(
                        u[:, :, 0:K], we_t[:], xin[:, u0 : u0 + HALF, 0:K],
                        start=False, stop=False,
                    )
                    nc.tensor.matmul(
                        u[:, :, W - K : W], we_t[:],
                        xin[:, u0 : u0 + HALF, W - K : W],
                        start=False, stop=True,
                    )
                    # interior block-cols: result is complete in PSUM;
                    # pure copy + f32->bf16 cast, split ScalarE (3 rows,
                    # closest to PSUM) / DVE (1 row) to balance engines
                    nc.scalar.copy(
                        out_t[:, u0 : u0 + HALF - 1, K : W - K],
                        u[:, 0 : HALF - 1, K : W - K],
                    )
                    nc.vector.tensor_copy(
                        out_t[:, u0 + HALF - 1, K : W - K],
                        u[:, HALF - 1, K : W - K],
                    )
                    # w-edge strips: out = gamma(partition) * PSUM
                    for w0 in (0, W - K):
                        nc.vector.tensor_scalar_mul(
                            out_t[:, u0 : u0 + HALF, w0 : w0 + K],
                            u[:, :, w0 : w0 + K],
                            gv_t[:],
                        )
                # stores exclusively on the second HWDGE ring
                # (qScalarDynamicHW) so loads and stores stream
                # concurrently without blocking each other
                nc.scalar.dma_start(y_v[p0 : p0 + P], out_t[:])
    nc.compile()
    return nc


_CACHE = {}


def _get_program():
    if "nc" not in _CACHE:
        _CACHE["nc"] = _build_program()
        _CACHE["w"] = _make_weights()
    return _CACHE["nc"], _CACHE["w"]


def run(x, trace=False, **spmd_kwargs):
    """x: [B, C, H, W] f32 -> (results object, output [B, C, H, W] f32)."""
    nc, weights = _get_program()
    x = np.ascontiguousarray(x, dtype=np.float32).astype(_BF16)
    in_maps = [{"x": x[i], **weights} for i in range(N_CORES)]
    res = run_bass_kernel_spmd(nc, in_maps, list(range(N_CORES)),
                               trace=trace, **spmd_kwargs)
    out = np.stack([res.results[i]["y"] for i in range(N_CORES)], axis=0)
    return res, out.astype(np.float32)


def kernel(x, kernel_size=8, **_ignored):
    assert int(kernel_size) == K, f"kernel hardcoded for k={K}"
    x = np.asarray(x)
    assert x.shape == (B, C, H, W), x.shape
    _, out = run(x)
    return out


if __name__ == "__main__":
    rng = np.random.default_rng(0)
    x = rng.standard_normal((B, C, H, W), dtype=np.float32)
    out = kernel(x, 8)
    print("out", out.shape, out.dtype, float(np.abs(out).mean()))


# revision 7
# speedup vs baseline: 1.0056x; 1.0056x over previous
"""Trainium2 Bass kernel for block-neighbor "contamination" stencil.

Problem: x [B=8, C=32, H=512, W=512] f32, kernel_size k=8.
The image is a 64x64 grid of 8x8 blocks. For each block, out = 0.8*block +
0.2 * mean(8 neighboring blocks) elementwise over the 8x8 tile, with
zero-padding of the block grid and per-position valid-neighbor counts
(interior 8, edges 5, corners 3).

Equivalent pixel form: a sparse 3x3 stencil with taps at +-8 pixels:
    out[r,w] = 0.8*x[r,w] + beta(r,w) * nsum[r,w]
    nsum[r,w] = sum over (dr,dw) in {-8,0,8}^2, (dr,dw) != (0,0), of
                x[r+dr, w+dw]  (zero pad at image borders)
    beta(r,w) = 0.2 / count(r,w),  count = Nr*Nw - 1,
    Nr/Nw = 2 at the first/last block row/col, else 3.

Strategy (pure data parallel, 1 batch item per NeuronCore, 8 cores).
The problem is HBM-bound (measured: DMA 92% busy at f32 I/O), so I/O is
done in bf16: the host casts x to bf16 (16 MiB/core in), the kernel
writes bf16 (16 MiB/core out), and the host upcasts to f32. End-to-end
rel err ~2.4e-3 (quantization), well within tolerance. This halves HBM
traffic: 32 MiB/core @ ~358 GB/s -> ~94 us floor vs 187 us at f32.

  * Layout: SBUF partition p = (channel-pair, block-row bi); free dim =
    (u = row-within-block 0..7, w 0..511). One partition = one block-row =
    8 consecutive image rows = 8KB contiguous DRAM at bf16.
  * Vertical block-neighbor taps (rows +-8) land at partition +-1 ->
    TensorEngine banded 128x128 matmuls (block-diagonal per channel),
    PSUM f32 accumulation. Horizontal taps are +-8 shifts along w via
    shifted moving-operand access patterns. beta(row) folded into the
    stationary weights; the center (0,0) tap with weight 0.8 is ALSO
    folded into the unshifted stationary (wc), so PSUM holds the full
    result for interior block-cols and the combine is a pure copy.
  * The copy PSUM->SBUF(bf16) runs on ScalarE (sits closest to PSUM;
    VectorE from PSUM is 1 elem/cycle and would become the bottleneck).
  * w-edge block-cols (first/last 8 columns) need the neighbor term
    rescaled by gamma(row) = (3Nr-1)/(2Nr-1): tiny diagonal matmuls
    (we) accumulate -0.8*(gamma-1)/gamma * x into the edge columns of
    PSUM, so out_edge = gamma * PSUM there -- one per-partition-scaled
    DVE copy (tensor_scalar_mul) per strip.
  * Matmuls are batched per stationary (4x wc, then 8x wf, then 8x we
    per half-tile) so LoadStationary switches hide under the PE's
    reorder window.
  * Input DMAs ride the qSyncDynamicHW ring, output DMAs the
    qScalarDynamicHW ring, so loads and stores stream concurrently.
"""

import numpy as np

import concourse.mybir as mybir
import concourse.tile as tile
from concourse import bacc
from concourse.bass_utils import run_bass_kernel_spmd

import ml_dtypes

_BF16 = ml_dtypes.bfloat16

# Problem constants (hardcoded per harness contract).
B, C, H, W = 8, 32, 512, 512
K = 8  # block size
P = 128  # SBUF partitions
NBR = H // K  # 64 block-rows per channel
CPP = P // NBR  # channels per partition-tile (2)
N_CORES = 8
N_CHUNKS = C // CPP  # 16 tiles per core
HALF = K // 2  # u-slices per PSUM tile (4 banks)

BETA_INT = 0.2 / 8.0  # interior block-rows, interior block-cols
BETA_EDGE = 0.2 / 5.0  # edge block-rows, interior block-cols
GAMMA_INT = 8.0 / 5.0  # count ratio (3*Nr-1)/(2*Nr-1) at Nr=3
GAMMA_EDGE = 5.0 / 3.0  # at Nr=2

_EDGE_PARTS = (0, NBR - 1, NBR, P - 1)  # block-row 0/63 of each channel


def _make_weights():
    """Banded stationary matrices (vertical taps at partition +-1),
    block-diagonal per channel, beta folded in per output partition.
    wc additionally carries the 0.8 center tap; we carries the per-row
    edge-column correction -0.8*(gamma-1)/gamma on the diagonal."""
    beta = np.full(P, BETA_INT, np.float32)
    beta[list(_EDGE_PARTS)] = BETA_EDGE
    gamma = np.full(P, GAMMA_INT, np.float32)
    gamma[list(_EDGE_PARTS)] = GAMMA_EDGE

    wf = np.zeros((P, P), np.float32)  # 3-band: taps for shifted columns
    wc = np.zeros((P, P), np.float32)  # 2-band + 0.8 center: unshifted
    for m in range(P):
        for d in (-1, 0, 1):
            k = m + d
            if 0 <= k < P and k // NBR == m // NBR:
                wf[k, m] = beta[m]
                if d != 0:
                    wc[k, m] = beta[m]
        wc[m, m] = 0.8
    we = np.diag(-0.8 * (gamma - 1.0) / gamma).astype(np.float32)
    gv = gamma.reshape(P, 1)
    return {
        "wf": wf.astype(_BF16),
        "wc": wc.astype(_BF16),
        "we": we.astype(_BF16),
        "gv": gv,
    }


def _build_program():
    f32 = mybir.dt.float32
    bf16 = mybir.dt.bfloat16

    nc = bacc.Bacc("TRN2", target_bir_lowering=False, debug=False,
                   num_devices=N_CORES)

    x_dram = nc.dram_tensor("x", [C, H, W], bf16, kind="ExternalInput")
    y_dram = nc.dram_tensor("y", [C, H, W], bf16, kind="ExternalOutput")
    wf_dram = nc.dram_tensor("wf", [P, P], bf16, kind="ExternalInput")
    wc_dram = nc.dram_tensor("wc", [P, P], bf16, kind="ExternalInput")
    we_dram = nc.dram_tensor("we", [P, P], bf16, kind="ExternalInput")
    gv_dram = nc.dram_tensor("gv", [P, 1], f32, kind="ExternalInput")

    # partition axis = (channel, block-row); free = (u, w)
    x_v = x_dram[:].rearrange("c (bi u) w -> (c bi) u w", u=K)
    y_v = y_dram[:].rearrange("c (bi u) w -> (c bi) u w", u=K)

    with tile.TileContext(nc) as tc:
        with (
            tc.tile_pool(name="wpool", bufs=1) as wpool,
            tc.tile_pool(name="sbuf", bufs=4) as sbuf,
            tc.tile_pool(name="psum", bufs=2, space="PSUM") as psum,
        ):
            # weight loads ride the scalar (store) ring, which is idle
            # at startup -- keeps the sync ring free so chunk-0's load
            # triggers immediately (weight DMAs ahead of it cost ~2.7us
            # of time-to-first-matmul otherwise)
            wf_t = wpool.tile([P, P], bf16, tag="wf")
            nc.scalar.dma_start(wf_t[:], wf_dram[:])
            wc_t = wpool.tile([P, P], bf16, tag="wc")
            nc.scalar.dma_start(wc_t[:], wc_dram[:])
            we_t = wpool.tile([P, P], bf16, tag="we")
            nc.scalar.dma_start(we_t[:], we_dram[:])
            gv_t = wpool.tile([P, 1], f32, tag="gv")
            nc.scalar.dma_start(gv_t[:], gv_dram[:])

            for i in range(N_CHUNKS):
                p0 = i * P
                xin = sbuf.tile([P, K, W], bf16, tag="xin")
                # loads stay exclusively on the qSyncDynamicHW ring:
                # mixing dependent stores into the same FIFO ring
                # head-of-line-blocks later loads
                if i == 0:
                    # split the pipeline-filling first load so half-0
                    # matmuls start ~2.4us sooner
                    nc.sync.dma_start(xin[:, 0:HALF, :],
                                      x_v[p0 : p0 + P, 0:HALF])
                    nc.sync.dma_start(xin[:, HALF:K, :],
                                      x_v[p0 : p0 + P, HALF:K])
                else:
                    nc.sync.dma_start(xin[:], x_v[p0 : p0 + P])

                out_t = sbuf.tile([P, K, W], bf16, tag="out")
                for h in range(2):
                    u0 = h * HALF
                    u = psum.tile([P, HALF, W], f32, tag="u")
                    # batched per stationary: wc x4, wf x8, we x8.
                    # each PSUM bank (one per uu) gets start on its wc
                    # matmul and stop on its last we matmul.
                    for uu in range(HALF):
                        # center 0.8 tap + vertical taps, full width
                        nc.tensor.matmul(
                            u[:, uu, :], wc_t[:], xin[:, u0 + uu, :],
                            start=True, stop=False,
                        )
                    for uu in range(HALF):
                        su = u0 + uu
                        # dj=-1 / dj=+1 neighbor block-columns
                        nc.tensor.matmul(
                            u[:, uu, K:W], wf_t[:], xin[:, su, : W - K],
                            start=False, stop=False,
                        )
                        nc.tensor.matmul(
                            u[:, uu, : W - K], wf_t[:], xin[:, su, K:W],
                            start=False, stop=False,
                        )
                    # edge-column correction so out_edge = gamma*PSUM;
                    # one matmul per side covers all 4 u-slices via a
                    # 2-D moving access pattern (FD=32)
                    nc.tensor.matmul(
                        u[:, :, 0:K], we_t[:], xin[:, u0 : u0 + HALF, 0:K],
                        start=False, stop=False,
                    )
                    nc.tensor.matmul(
                        u[:, :, W - K : W], we_t[:],
                        xin[:, u0 : u0 + HALF, W - K : W],
                        start=False, stop=True,
                    )
                    # interior block-cols: result is complete in PSUM;
                    # pure copy + f32->bf16 cast, split ScalarE (3 rows,
                    # closest to PSUM) / DVE (1 row) to balance engines
                    nc.scalar.copy(
                        out_t[:, u0 : u0 + HALF - 1, K : W - K],
                        u[:, 0 : HALF - 1, K : W - K],
                    )
                    nc.vector.tensor_copy(
                        out_t[:, u0 + HALF - 1, K : W - K],
                        u[:, HALF - 1, K : W - K],
                    )
                    # w-edge strips: out = gamma(partition) * PSUM
                    for w0 in (0, W - K):
                        nc.vector.tensor_scalar_mul(
                            out_t[:, u0 : u0 + HALF, w0 : w0 + K],
                            u[:, :, w0 : w0 + K],
                            gv_t[:],
                        )
                    if i == N_CHUNKS - 1:
                        # drain the pipeline tail per half so the final
                        # store (and its ~2us completion receipt) starts
                        # as early as possible
                        nc.scalar.dma_start(
                            y_v[p0 : p0 + P, u0 : u0 + HALF],
                            out_t[:, u0 : u0 + HALF],
                        )
                # stores exclusively on the second HWDGE ring
                # (qScalarDynamicHW) so loads and stores stream
                # concurrently without blocking each other
                if i != N_CHUNKS - 1:
                    nc.scalar.dma_start(y_v[p0 : p0 + P], out_t[:])
    nc.compile()
    return nc


_CACHE = {}


def _get_program():
    if "nc" not in _CACHE:
        _CACHE["nc"] = _build_program()
        _CACHE["w"] = _make_weights()
    return _CACHE["nc"], _CACHE["w"]


def run(x, trace=False, **spmd_kwargs):
    """x: [B, C, H, W] f32 -> (results object, output [B, C, H, W] f32)."""
    nc, weights = _get_program()
    x = np.ascontiguousarray(x, dtype=np.float32).astype(_BF16)
    in_maps = [{"x": x[i], **weights} for i in range(N_CORES)]
    res = run_bass_kernel_spmd(nc, in_maps, list(range(N_CORES)),
                               trace=trace, **spmd_kwargs)
    out = np.stack([res.results[i]["y"] for i in range(N_CORES)], axis=0)
    return res, out.astype(np.float32)


def kernel(x, kernel_size=8, **_ignored):
    assert int(kernel_size) == K, f"kernel hardcoded for k={K}"
    x = np.asarray(x)
    assert x.shape == (B, C, H, W), x.shape
    _, out = run(x)
    return out


if __name__ == "__main__":
    rng = np.random.default_rng(0)
    x = rng.standard_normal((B, C, H, W), dtype=np.float32)
    out = kernel(x, 8)
    print("out", out.shape, out.dtype, float(np.abs(out).mean()))
